# revision 8
# baseline (speedup 1.0000x reference)
"""Trainium2 Bass kernel for nn_GAttention (gnn_message_passing).

Computation (per batch b):
    k  = einsum('cnt,c->nt', x[b], alpha)
    kG = k @ Gw
    S  = kG @ k.T                  # [N, N]
    att = softmax(S, axis=-1)      # rows
    out[b] = einsum('nm,cmt->cnt', att * adj, x[b])

Sharding: data-parallel over batch B=16 across 8 cores (2 batches/core).
adj/Gw/alpha replicated. No collectives.

Device-side layout strategy (per batch):
  - x loaded transposed as xT[m, (c,t)] tiles (m on partitions) via strided
    DMA; cast to bf16 for the aggregation matmul; k computed from the fp32
    tiles on DVE (alpha-weighted tree reduction over c).
  - scores computed TRANSPOSED: ST[m, n] = k[m] . kG[n] so that the
    softmax-masked weights come out in the [m, n] layout the aggregation
    matmul needs as its stationary operand (contraction over m).
  - softmax without max-subtraction (scores are O(+-15), exp is safe in
    fp32); denominator = column sums of exp(ST), computed with a
    ones-vector matmul trick accumulated into a single PSUM bank.
  - aggregation: out2[n, (c,t)] = sum_m wT[m, n] * xT[m, (c,t)] in bf16,
    PSUM fp32 accumulation over 16 m-tiles; evicted through ScalarE with
    per-partition 1/denom scale, stored back with strided DMA.
"""

import functools

import numpy as np

import concourse.bass as bass
import concourse.bacc as bacc
import concourse.mybir as mybir
import concourse.tile as tile
from concourse.bass_utils import run_bass_kernel_spmd
from concourse.masks import make_identity

# Problem shape (hardcoded per contract).
B, C, N, T = 16, 64, 2048, 24
NCORES = 8
BPC = B // NCORES            # batches per core
P = 128                      # partitions
CT = C * T                   # 1536
NT = N // P                  # 16 n/m tiles
NHALF = 1024                 # n processed in halves (SBUF budget for wT)
NTL = NHALF // P             # 8 n-tiles per half
F32 = mybir.dt.float32
F32R = mybir.dt.float32r     # fp32 storage, single-pass PE multiply
BF16 = mybir.dt.bfloat16


def ts(i, sz):
    return bass.ts(i, sz)


def _build_kernel_body(tc: tile.TileContext, x, adjt, gw, alpha, out):
    nc = tc.nc
    ctx_pools = []

    def pool(name, bufs, space="SBUF"):
        p = tc.alloc_tile_pool(name=name, bufs=bufs, space=space)
        ctx_pools.append(p)
        return p

    singles = pool("singles", 1)
    adjp = pool("adjp", NT)          # 16 resident bf16 adjT tiles
    xfp = pool("xf", 2)              # fp32 x staging (strided loads land here)
    xbp = pool("xb", NT)             # 16 resident bf16 xT tiles (per batch)
    kp = pool("kp", 2)               # k [128, 16, 24] per batch
    ktp = pool("ktp", 1)             # kT [24, 2048] f32r per batch
    kgp = pool("kgp", 1)             # kGT [24, 2048] f32r per batch
    ep = pool("ep", 8)              # exp(ST) bf16 chunks
    wtp = pool("wtp", NT)            # 16 wT half-tiles [128, 1024] bf16
    osbp = pool("osb", 2)            # output staging fp32
    rcp = pool("rcp", 2)             # reciprocal denominators
    ps_st = pool("ps_st", 1, space="PSUM")   # scores / small matmuls
    ps_dn = pool("ps_dn", 1, space="PSUM")   # denominator bank
    ps_o = pool("ps_o", 2, space="PSUM")     # aggregation accumulators

    # --- one-time setup ---------------------------------------------------
    ident = singles.tile([P, P], F32)
    make_identity(nc, ident)

    alpha_rep = singles.tile([P, C], F32)
    nc.gpsimd.dma_start(
        out=alpha_rep,
        in_=bass.AP(tensor=alpha.tensor, offset=0, ap=[[0, P], [1, C]]),
    )

    gw_sb = singles.tile([T, T], F32R)
    nc.gpsimd.dma_start(out=gw_sb, in_=gw[:, :])

    # e_q stationaries for the denominator trick: column q is ones.
    eq_tiles = []
    for q in range(2):
        e_q = singles.tile([P, 2], BF16, name=f"eq{q}")
        nc.vector.memset(e_q, 0.0)
        nc.vector.memset(e_q[:, q : q + 1], 1.0)
        eq_tiles.append(e_q)

    # adjT resident in bf16 (cast during SWDGE DMA).
    adjt_bf = []
    for mt in range(NT):
        t_ = adjp.tile([P, N], BF16, name=f"adjt{mt}", tag="adjt")
        nc.gpsimd.dma_start(out=t_, in_=adjt[ts(mt, P), :])
        adjt_bf.append(t_)

    # --- per batch --------------------------------------------------------
    for b in range(BPC):
        x_b = x[b].rearrange("c (mo p) t -> mo p c t", p=P)      # [16,128,C,T]
        out_b = out[b].rearrange("c (no p) t -> no p c t", p=P)  # [16,128,C,T]

        # Phase 0: load xT tiles, cast to bf16, compute k, kT, kGT.
        xb_tiles = []
        k_all = kp.tile([P, NT, T], F32, name="k_all")
        for mt in range(NT):
            xf = xfp.tile([P, CT], F32, name="xf")
            xf3 = xf.rearrange("p (c t) -> p c t", t=T)
            nc.sync.dma_start(out=xf3, in_=x_b[mt])

            xb_t = xbp.tile([P, CT], BF16, name="xb")
            nc.vector.tensor_copy(out=xb_t, in_=xf)
            xb_tiles.append(xb_t)

            # k[m, t] = sum_c alpha[c] * x[c, m, t]  (in-place on xf)
            nc.vector.tensor_tensor(
                xf3,
                xf3,
                alpha_rep[:, :, None].to_broadcast((P, C, T)),
                mybir.AluOpType.mult,
            )
            s = C // 2
            while s >= 1:
                nc.vector.tensor_add(
                    out=xf3[:, :s, :], in0=xf3[:, :s, :], in1=xf3[:, s : 2 * s, :]
                )
                s //= 2
            nc.vector.tensor_copy(out=k_all[:, mt, :], in_=xf3[:, 0, :])

        # kT [24, 2048] via PE transposes of k tiles.
        kt_sb = ktp.tile([T, N], F32R, name="kt")
        for mt in range(NT):
            ps = ps_st.tile([P, 512], F32, name="st")
            nc.tensor.transpose(ps[:T, :P], k_all[:, mt, :], ident)
            nc.vector.tensor_copy(out=kt_sb[:, ts(mt, P)], in_=ps[:T, :P])

        # kGT[s, n] = sum_t Gw[t, s] * kT[t, n]
        kgt_sb = kgp.tile([T, N], F32R, name="kgt")
        for q4 in range(N // 512):
            ps = ps_st.tile([P, 512], F32, name="st")
            nc.tensor.matmul(
                ps[:T, :512], gw_sb, kt_sb[:, ts(q4, 512)], start=True, stop=True
            )
            nc.vector.tensor_copy(out=kgt_sb[:, ts(q4, 512)], in_=ps[:T, :512])

        # Phases 1+2 per n-half.
        for h in range(2):
            # Phase 1: ST -> exp -> denom -> wT(half)
            dn = ps_dn.tile([2, 512], F32, name="dn")
            wt_tiles = []
            for mt in range(NT):
                wt_t = wtp.tile([P, NHALF], BF16, name="wt")
                wt_tiles.append(wt_t)
                for q in range(2):
                    nsl = slice(h * NHALF + q * 512, h * NHALF + (q + 1) * 512)
                    st_t = ps_st.tile([P, 512], F32, name="st")
                    nc.tensor.matmul(
                        st_t,
                        kt_sb[:, ts(mt, P)],
                        kgt_sb[:, nsl],
                        start=True,
                        stop=True,
                    )
                    e_t = ep.tile([P, 512], BF16, name="e")
                    nc.scalar.activation(
                        out=e_t, in_=st_t, func=mybir.ActivationFunctionType.Exp
                    )
                    nc.tensor.matmul(
                        dn,
                        eq_tiles[q],
                        e_t,
                        start=(mt == 0 and q == 0),
                        stop=(mt == NT - 1 and q == 1),
                    )
                    nc.vector.tensor_mul(
                        out=wt_t[:, ts(q, 512)],
                        in0=e_t,
                        in1=adjt_bf[mt][:, nsl],
                    )

            # 1/denominator, scattered to per-partition layout
            # recipT[p, j] = 1 / denom[n = h*1024 + j*128 + p], j = 0..7
            recip_sb = rcp.tile([2, 512], F32, name="recip")
            nc.vector.reciprocal(out=recip_sb, in_=dn)
            recip_t = rcp.tile([P, NTL], F32, name="recipt")
            for j in range(NTL):
                q, j4 = divmod(j, 4)
                nc.sync.dma_start(
                    out=recip_t[:, j : j + 1],
                    in_=recip_sb[q : q + 1, j4 * P : (j4 + 1) * P],
                )

            # Phase 2: aggregation over m for each n-tile of this half.
            for ntl in range(NTL):
                nt_g = h * NTL + ntl
                o_t = ps_o.tile([P, CT], F32, name="o")
                for mt in range(NT):
                    for ch in range(3):
                        nc.tensor.matmul(
                            o_t[:, ts(ch, 512)],
                            wt_tiles[mt][:, ts(ntl, P)],
                            xb_tiles[mt][:, ts(ch, 512)],
                            start=(mt == 0),
                            stop=(mt == NT - 1),
                        )
                osb = osbp.tile([P, CT], F32, name="osb")
                for ch in range(3):
                    nc.scalar.activation(
                        out=osb[:, ts(ch, 512)],
                        in_=o_t[:, ts(ch, 512)],
                        func=mybir.ActivationFunctionType.Copy,
                        scale=recip_t[:, ntl : ntl + 1],
                    )
                nc.sync.dma_start(
                    out=out_b[nt_g], in_=osb.rearrange("p (c t) -> p c t", t=T)
                )

    for p_ in reversed(ctx_pools):
        p_.release()


@functools.lru_cache(maxsize=1)
def _build_nc():
    nc = bacc.Bacc(trn_type="TRN2")
    x = nc.dram_tensor("x", [BPC, C, N, T], F32, kind="ExternalInput")
    adjt = nc.dram_tensor("adjt", [N, N], F32, kind="ExternalInput")
    gw = nc.dram_tensor("gw", [T, T], F32, kind="ExternalInput")
    alpha = nc.dram_tensor("alpha", [C], F32, kind="ExternalInput")
    out = nc.dram_tensor("out", [BPC, C, N, T], F32, kind="ExternalOutput")
    with tile.TileContext(nc) as tc:
        _build_kernel_body(tc, x[:], adjt[:], gw[:], alpha[:], out[:])
    nc.finalize()
    return nc


def run(x, adj, Gw, alpha, trace=False):
    nc = _build_nc()
    x = np.ascontiguousarray(x, dtype=np.float32)
    adjt = np.ascontiguousarray(np.asarray(adj, dtype=np.float32).T)
    gw = np.ascontiguousarray(Gw, dtype=np.float32)
    al = np.ascontiguousarray(alpha, dtype=np.float32)
    in_maps = [
        {"x": x[i * BPC : (i + 1) * BPC], "adjt": adjt, "gw": gw, "alpha": al}
        for i in range(NCORES)
    ]
    res = run_bass_kernel_spmd(nc, in_maps, list(range(NCORES)), trace=trace)
    outv = np.concatenate([r["out"] for r in res.results], axis=0)
    return outv, res


def kernel(x, adj, Gw, alpha):
    outv, _ = run(x, adj, Gw, alpha, trace=False)
    return outv
